# revision 40
# baseline (speedup 1.0000x reference)
"""Trainium2 Bass kernel for AtomicNumberPooling (segment-sum pooling).

Math: output[b, (z-1)*F + f] = sum_{n: batch[n]==b, z[n]==z} out[n, f],
i.e. a segment sum over combined id seg = batch*100 + (z-1), reshaped to
[B, 100*F].

Strategy (v3)
-------------
`batch` is sorted, so sharding the B=1000 graphs contiguously over 8 cores
gives each core a contiguous row range of `out` and a fully disjoint slice
of the output - no collective needed.

Host-side packing (index bookkeeping + dtype casts only):
  * rows are grouped per graph and zero-padded to GPAD=256 rows/graph
    (2 tiles of 128; the real max for this input size is 252; overflow
    rows fall back to a host-side correction);
  * x is shipped as fp16 (256B/row) laid out partition-major [128, NT*F];
  * the z one-hot is built ON HOST as fp8_e4m3 [128, NT*100] (values 0/1
    are exact in e4m3; 100B/row); padding rows get all-zero one-hot rows.

Device program (per core, identical SPMD):
  * PE: one matmul per 128-row tile, accumulating into a per-graph psum
    column slot (5 slots of 100 f32 per 2KB bank, 48B hole per bank).
    Main blocks rotate over NPS psum tensors; the last TAIL_N small
    blocks get dedicated 1-bank tensors so the tail never waits on the
    saturated main copy chain.
  * Psum evacuation: strided copy psum->osb fp16 per block of graphs,
    mostly on DVE (the only non-DMA-capable engine that can read psum;
    it is the pacing resource: ~13us serial). A late block plus the last
    tail block go to ACT (which prepays its 1383ns activation-table load
    mid-stream) so the final copies drain on two engines in parallel.
  * The three DMA queues (SP=sync, ACT=scalar, Pool=gpsimd) are balanced
    by a greedy water-fill planner using the cost model (0.3855 ns per
    free-dim byte, serialized per queue, parallel across queues, 500ns
    floor per DMA, 2x penalty below 512B contiguous): all loads first
    (minimizes the last-load time that gates the matmul/copy chain),
    then stores appended in block order to the least-loaded queue.
  * raw bass Block with explicit single-semaphore waits; one semaphore
    per load chunk (DMA completions may land out of order, even within
    a queue). Semaphore clears run in the per-execution PROLOG (before
    the start barrier), so there is no trailing cleanup.
"""

from contextlib import ExitStack

import ml_dtypes
import numpy as np

import concourse.bass as bass
import concourse.mybir as mybir
from concourse.bass_utils import run_bass_kernel_spmd

NCORES = 8
B = 1000
MAX_Z = 100
F = 128
TP = 128                 # rows per tile (SBUF partition dim)
GB = B // NCORES         # graphs per core
GPAD = 256               # padded rows per graph (real max for this input: 252)
SPG = GPAD // TP         # row tiles per graph (= 2)
NT = GB * SPG            # row tiles per core (= 250)
RPC = GB * GPAD          # padded rows per core (= 32000)
E4M3 = ml_dtypes.float8_e4m3

# psum layout: 5 graph slots of 100 f32 per 2KB bank (48B hole).
# Main blocks rotate over NPS tensors of BANKS_PER_PS banks; the last
# TAIL_N blocks (each <= 5 graphs) get DEDICATED 1-bank tensors so their
# matmuls/copies/stores don't wait on the saturated main copy chain.
# Constraint: NPS * BANKS_PER_PS + TAIL_N <= 8 banks.
SLOTS_PER_BANK = 5
BANKS_PER_PS = 2
NPS = 3
TAIL_N = 2

# -- schedule knobs ---------------------------------------------------------
# small first block -> copy chain starts early; small tail blocks -> short
# end-of-pipeline copy+store chain. Sizes must be <5 or a multiple of 5,
# and at most 5*BANKS_PER_PS.
BLOCK_SIZES = [5] + [10] * 11 + [5, 5]          # sums to GB = 125
# copy engine per block: DVE for the bulk; ACT steals DMA capacity AND pays
# a one-time 1383ns activation-table load, so use it sparingly for the tail
COPY_ENG = ["dve"] * 11 + ["act", "dve", "act"]
ACT_PREPAY = True      # pay the ACT table load mid-stream instead of on tail

NS_PER_B = 0.3855      # cost-model DMA ns per free-dim byte
X_TILE_B = F * 2       # 256 B
OH_TILE_B = MAX_Z      # 100 B


def _dma_cost(nbytes, elem):
    mult = 2.0 if elem < 512 else 1.0
    return max(nbytes * NS_PER_B * mult, 500.0)


def _blocks():
    out = []
    g = 0
    for w in BLOCK_SIZES:
        out.append((g, w))
        g += w
    assert g == GB
    return out


def _tail0():
    return len(BLOCK_SIZES) - TAIL_N


def _psum_col(g_in_block):
    return 512 * (g_in_block // SLOTS_PER_BANK) + 100 * (g_in_block % SLOTS_PER_BANK)


def _load_schedule():
    """Greedy balanced schedule. Returns {queue: [items]} with items
    ("x"|"oh", tile_start, ntiles), ("st", first_block, last_block) or
    ("prepay",), in per-queue program order.

    All loads are emitted first (balanced water-fill; minimizes the time of
    the last load, which gates the matmul/copy chain), then all stores are
    appended in block order to whichever queue is least loaded."""
    blocks = _blocks()
    nblk = len(blocks)
    q = {"sp": [], "act": [], "pool": []}
    clock = {"sp": 0.0, "act": 0.0, "pool": 0.0}
    # Pool's DMA init is ~170ns later than SP/ACT; bias its clock so the
    # greedy doesn't overfill it
    clock["pool"] = 170.0

    def put(queue, item, cost):
        q[queue].append(item)
        clock[queue] += cost

    if ACT_PREPAY and any(e == "act" for e in COPY_ENG):
        prepay_pending = True
    else:
        prepay_pending = False

    for k, (g0, gw) in enumerate(blocks):
        t0, tn = g0 * SPG, gw * SPG
        if k == 0:
            # fast start: block 0 split across all three queues
            half = tn // 2
            put("sp", ("x", t0, half), _dma_cost(half * X_TILE_B, half * X_TILE_B))
            put("act", ("x", t0 + half, tn - half),
                _dma_cost((tn - half) * X_TILE_B, (tn - half) * X_TILE_B))
            put("pool", ("oh", t0, tn), _dma_cost(tn * OH_TILE_B, tn * OH_TILE_B))
            continue
        # one-hot chunk: whole block, to the cheaper of act/pool
        ohq = "act" if clock["act"] <= clock["pool"] else "pool"
        put(ohq, ("oh", t0, tn), _dma_cost(tn * OH_TILE_B, tn * OH_TILE_B))
        if prepay_pending and k >= 2:
            put("act", ("prepay",), 1575.0)
            prepay_pending = False
        # x tiles: water-fill across all three queues (min chunk 4 tiles)
        rem = tn
        pos = t0
        total = sum(clock.values()) + tn * X_TILE_B * NS_PER_B
        target = total / 3.0
        shares = []
        for queue in sorted(clock, key=clock.get):
            want = max(0.0, target - clock[queue])
            ntl = int(round(want / (X_TILE_B * NS_PER_B)))
            ntl = min(ntl, rem)
            shares.append((queue, ntl))
            rem -= ntl
        if rem > 0:
            queue, ntl = shares[0]
            shares[0] = (queue, ntl + rem)
            rem = 0
        # merge tiny chunks into the largest share
        shares = [(qq, n) for qq, n in shares if n > 0]
        if len(shares) > 1:
            shares.sort(key=lambda s: -s[1])
            while len(shares) > 1 and shares[-1][1] < 4:
                qq, n = shares.pop()
                q0, n0 = shares[0]
                shares[0] = (q0, n0 + n)
        for queue, ntl in shares:
            put(queue, ("x", pos, ntl), _dma_cost(ntl * X_TILE_B, ntl * X_TILE_B))
            pos += ntl
        assert pos == t0 + tn
    # stores: all after the loads. Early blocks (copies long done) first,
    # round-robin to the least-loaded queue; the last few blocks' stores are
    # gated by the tail of the copy chain, so they go last — and ACT's tail
    # copies are placed before them in ACT's stream (via "cp" items).
    act_copy_blocks = [k for k in range(nblk) if COPY_ENG[k] == "act"]
    late0 = min([nblk - 3] + [k - 1 for k in act_copy_blocks])
    # ACT's tail copies occupy ACT during the store phase: charge them now
    # (NOT during load placement) so ACT gets fewer stores
    for k in act_copy_blocks:
        clock["act"] += blocks[k][1] * MAX_Z * 0.833 + 185
    for k in range(nblk):
        if k < late0:
            gw = blocks[k][1]
            nb = gw * MAX_Z * 2
            queue = min(clock, key=clock.get)
            put(queue, ("st", k, k), _dma_cost(nb, nb))
    for k in act_copy_blocks:
        put("act", ("cp", k), blocks[k][1] * MAX_Z * 0.833 + 185)
    for k in range(late0, nblk):
        gw = blocks[k][1]
        nb = gw * MAX_Z * 2
        queue = min(clock, key=clock.get)
        put(queue, ("st", k, k), _dma_cost(nb, nb))
    return q


def _build(start_clear=True):
    blocks = _blocks()
    nblk = len(blocks)
    plan = _load_schedule()

    # tile -> (block, graph-in-block, start, stop)
    tile_info = []
    for t in range(NT):
        g, s = divmod(t, SPG)
        k = next(i for i, (g0, gw) in enumerate(blocks) if g0 <= g < g0 + gw)
        tile_info.append((k, g - blocks[k][0], s == 0, s == SPG - 1))
    blk_mm_done = [(blocks[k][0] + blocks[k][1]) * SPG for k in range(nblk)]

    nc = bass.Bass()
    x = nc.dram_tensor("x", [TP, NT * F], mybir.dt.float16, kind="ExternalInput")
    oh = nc.dram_tensor("oh", [TP, NT * MAX_Z], mybir.dt.float8e4,
                        kind="ExternalInput")
    o = nc.dram_tensor("o", [TP, GB * MAX_Z], mybir.dt.float16,
                       kind="ExternalOutput")

    with ExitStack() as ctx:
        xb = ctx.enter_context(
            nc.sbuf_tensor("xb", [TP, NT * F], mybir.dt.float16))
        ohb = ctx.enter_context(
            nc.sbuf_tensor("ohb", [TP, NT * MAX_Z], mybir.dt.float8e4))
        osb = ctx.enter_context(
            nc.sbuf_tensor("osb", [TP, GB * MAX_Z], mybir.dt.float16))
        scr = ctx.enter_context(
            nc.sbuf_tensor("scr", [TP, 8], mybir.dt.float16))
        ps = [
            ctx.enter_context(
                nc.psum_tensor(f"ps{i}", [TP, 512 * BANKS_PER_PS],
                               mybir.dt.float32))
            for i in range(NPS)
        ]
        ps_tail = [
            ctx.enter_context(
                nc.psum_tensor(f"pst{i}", [TP, 512], mybir.dt.float32))
            for i in range(TAIL_N)
        ]

        def ps_of(k):
            if k >= _tail0():
                return ps_tail[k - _tail0()]
            return ps[k % NPS]

        # one semaphore per load chunk (DMA completions may land out of
        # order, even within one queue)
        n_load_chunks = sum(
            1 for queue in ("sp", "act", "pool") for item in plan[queue]
            if item[0] in ("x", "oh"))
        s_ld = [ctx.enter_context(nc.semaphore(f"s_ld{i}"))
                for i in range(n_load_chunks)]
        s_mm = ctx.enter_context(nc.semaphore("s_mm"))   # +1 per tile matmul
        s_cpb = [ctx.enter_context(nc.semaphore(f"s_cpb{i}"))
                 for i in range(nblk)]                   # +1 per block copy
        s_st = {q: ctx.enter_context(nc.semaphore(f"s_st_{q}"))
                for q in ("pool", "sp", "act")}
        my_sems = [*s_ld, s_mm, *s_cpb, *s_st.values()]

        chunk_sem = {}           # ("x"|"oh", a, w) -> sem index
        si = 0
        for queue in ("sp", "act", "pool"):
            for item in plan[queue]:
                if item[0] in ("x", "oh"):
                    chunk_sem[item] = si
                    si += 1
        tile_xwait = [None] * NT
        tile_ohwait = [None] * NT
        for (kind, a, w), j in chunk_sem.items():
            for t in range(a, a + w):
                if kind == "x":
                    tile_xwait[t] = j
                else:
                    tile_ohwait[t] = j

        def emit_copy(eng, k):
            g0, gw = blocks[k]
            eng.wait_ge(s_mm, blk_mm_done[k])
            if gw % SLOTS_PER_BANK == 0:
                nbank = gw // SLOTS_PER_BANK
                src = ps_of(k)[:, 0:512 * nbank].rearrange(
                    "p (b c) -> p b c", c=512)[:, :, 0:500]
                dst = osb[:, g0 * MAX_Z:(g0 + gw) * MAX_Z].rearrange(
                    "p (b c) -> p b c", c=500)
            else:
                assert gw < SLOTS_PER_BANK
                src = ps_of(k)[:, 0:gw * MAX_Z]
                dst = osb[:, g0 * MAX_Z:(g0 + gw) * MAX_Z]
            if hasattr(eng, "tensor_copy"):
                eng.tensor_copy(dst, src).then_inc(s_cpb[k], 1)
            elif hasattr(eng, "tensor_scalar_mul"):
                eng.tensor_scalar_mul(dst, src, 1.0).then_inc(s_cpb[k], 1)
            else:
                eng.copy(dst, src).then_inc(s_cpb[k], 1)

        def emit_store(eng, kfirst, klast, q):
            g0 = blocks[kfirst][0]
            g1 = blocks[klast][0] + blocks[klast][1]
            for k in range(kfirst, klast + 1):
                eng.wait_ge(s_cpb[k], 1)
            eng.dma_start(
                o[:, g0 * MAX_Z:g1 * MAX_Z],
                osb[:, g0 * MAX_Z:g1 * MAX_Z],
            ).then_inc(s_st[q], 16)

        def emit_queue(eng, queue):
            n_st = 0
            for item in plan[queue]:
                if item[0] == "prepay":
                    # one-time ACT activation-table load, off the tail chain
                    eng.wait_ge(s_ld[tile_xwait[0]], 16)
                    eng.copy(scr[:], xb[:, 0:8])
                elif item[0] == "cp":
                    emit_copy(eng, item[1])
                elif item[0] == "x":
                    _kind, a, w = item
                    eng.dma_start(
                        xb[:, a * F:(a + w) * F],
                        x[:, a * F:(a + w) * F],
                    ).then_inc(s_ld[chunk_sem[item]], 16)
                elif item[0] == "oh":
                    _kind, a, w = item
                    eng.dma_start(
                        ohb[:, a * MAX_Z:(a + w) * MAX_Z],
                        oh[:, a * MAX_Z:(a + w) * MAX_Z],
                    ).then_inc(s_ld[chunk_sem[item]], 16)
                else:
                    _kind, kf, kl = item
                    emit_store(eng, kf, kl, queue)
                    n_st += 1
            if n_st:
                eng.wait_ge(s_st[queue], 16 * n_st)

        if start_clear:
            # per-execution prolog: reset DGE queues and clear all our
            # semaphores BEFORE the start barrier (split across engines so
            # the prolog is short). The epilogue needs no clears: this
            # prolog re-runs on every execution of the NEFF.
            nc.gpsimd.dma_reset()
            engs = [nc.gpsimd, nc.sync, nc.scalar, nc.vector]
            for i, s in enumerate(my_sems):
                engs[i % 4].sem_clear(s)
            nc._nrt_pseudo_barrier()

        with nc.Block() as block:

            @block.sync
            def _(sync):
                emit_queue(sync, "sp")

            @block.scalar
            def _(scalar):
                emit_queue(scalar, "act")

            @block.gpsimd
            def _(gpsimd):
                emit_queue(gpsimd, "pool")

            @block.tensor
            def _(tensor):
                seen = set()

                def need(j):
                    if j not in seen:
                        tensor.wait_ge(s_ld[j], 16)
                        seen.add(j)

                for t in range(NT):
                    k, gq, st0, st1 = tile_info[t]
                    need(tile_xwait[t])
                    need(tile_ohwait[t])
                    if st0 and gq == 0 and k < _tail0() and k >= NPS:
                        tensor.wait_ge(s_cpb[k - NPS], 1)      # psum free
                    col = _psum_col(gq)
                    tensor.matmul(
                        ps_of(k)[:, col:col + MAX_Z],
                        xb[:, t * F:(t + 1) * F],
                        ohb[:, t * MAX_Z:(t + 1) * MAX_Z],
                        start=st0, stop=st1,
                    ).then_inc(s_mm, 1)

            @block.vector
            def _(vector):
                for k in range(nblk):
                    if COPY_ENG[k] == "dve":
                        emit_copy(vector, k)

        # Block exit emitted an all-engine barrier: everything is quiesced.
        # Semaphore cleanup happens in the next execution's prolog (above),
        # so no trailing work is needed here.

    return nc


def _sim_inputs():
    return [("x", (TP, NT * F), np.float16),
            ("oh", (TP, NT * MAX_Z), E4M3)]


_NC = None


def _get_nc():
    global _NC
    if _NC is None:
        _NC = _build()
    return _NC


def _pack_inputs(x, z, b):
    """Build per-core input maps; returns (in_maps, host_fix).

    host_fix is a [B*MAX_Z, F] float32 correction for rows that could not
    be placed on the device (graph overflow beyond GPAD) - all zeros for
    sane inputs; kept for robustness.
    """
    in_maps = []
    host_fix = None
    zcol = z.astype(np.int64) - 1
    x16 = x.astype(np.float16)
    for c in range(NCORES):
        g_lo, g_hi = c * GB, (c + 1) * GB
        r0 = np.searchsorted(b, g_lo, side="left")
        r1 = np.searchsorted(b, g_hi, side="left")
        bb = (b[r0:r1] - g_lo).astype(np.int64)
        zz = zcol[r0:r1]
        hh = x16[r0:r1]

        cnt = np.bincount(bb, minlength=GB)
        offs = np.zeros(GB + 1, np.int64)
        offs[1:] = np.cumsum(cnt)
        rank = np.arange(len(bb)) - offs[bb]

        zok = (zz >= 0) & (zz < MAX_Z)
        ok = (rank < GPAD) & zok
        if not (rank < GPAD).all():
            # overflow rows: accumulate on host (never hit for this dataset)
            if host_fix is None:
                host_fix = np.zeros((B * MAX_Z, F), np.float32)
            sel = (~(rank < GPAD)) & zok
            seg = (b[r0:r1][sel].astype(np.int64) * MAX_Z + zz[sel])
            np.add.at(host_fix, seg, x[r0:r1][sel])
        bb, zz, hh, rank = bb[ok], zz[ok], hh[ok], rank[ok]

        dest = bb * GPAD + rank
        xp = np.zeros((RPC, F), np.float16)
        xp[dest] = hh
        ohp = np.zeros((RPC, MAX_Z), E4M3)
        ohp[dest, zz] = E4M3(1.0)
        # partition-major: row r -> [r % 128, (r // 128)*W : ...]
        xm = np.ascontiguousarray(
            xp.reshape(NT, TP, F).transpose(1, 0, 2).reshape(TP, NT * F))
        ohm = np.ascontiguousarray(
            ohp.reshape(NT, TP, MAX_Z).transpose(1, 0, 2)
            .reshape(TP, NT * MAX_Z))
        in_maps.append({"x": xm, "oh": ohm})
    return in_maps, host_fix


def kernel(out, z, batch):
    x = np.asarray(out, dtype=np.float32)
    z = np.asarray(z)
    b = np.asarray(batch)

    if np.any(b[1:] < b[:-1]):                # robustness: ensure sorted
        order = np.argsort(b, kind="stable")
        x, z, b = x[order], z[order], b[order]
    valid = (b >= 0) & (b < B)                # out-of-range graphs: dropped
    if not valid.all():
        x, z, b = x[valid], z[valid], b[valid]

    in_maps, host_fix = _pack_inputs(x, z, b)
    res = run_bass_kernel_spmd(_get_nc(), in_maps, list(range(NCORES)))
    # device output is partition-major [F, GB*MAX_Z]; transpose to
    # [GB*MAX_Z, F] per core while gathering
    blocks = [
        np.ascontiguousarray(res.results[c]["o"].T).astype(np.float32)
        for c in range(NCORES)
    ]
    pooled = np.concatenate(blocks, axis=0)
    if host_fix is not None:
        pooled = pooled + host_fix
    return pooled.reshape(B, MAX_Z * F)
